# revision 13
# baseline (speedup 1.0000x reference)
"""Trainium2 Bass kernel for GeneralPolyGNN (B=128 graphs, N=512 nodes, F=128).

Model:  S = D^-1/2 A D^-1/2 (per-graph dense GSO from edge_index)
        x = relu(poly3(S, x) @ W1 + b1); x = relu(poly3(S, x) @ W2 + b2)
        h = mean_nodes(x); out = relu(h @ Wr1 + br1) @ Wr2 + br2
        where poly3(S, x) = x + Sx + S^2x + S^3x  (Horner: x + S(x + S(x + Sx)))

Sharding: data-parallel over graphs — 16 graphs per core on 8 NeuronCores.
The host shards node features and lays each graph's edge set out as its dense
0/1 transposed-adjacency block (partition-contiguous bf16); all numeric work
(degrees, normalization, polynomial matmul chain, activations, pooling, head
MLP) runs on device.

Device layout (per graph g):
  att[g, p, b*512 + i] = 1 iff edge i -> j, j = b*128 + p      (A^T as bf16)
  x tiles: [128 part, (4 node-blocks x 128 feat)], node = blk*128 + p
  Sb = bf16(dinv_j * A^T * dinv_i): scaled GSO transpose, built on device
  apply: t[ib] += Sb[jb, ib-slice].T @ z[jb]   (PSUM f32, 16 matmuls)
"""

import os
import numpy as np
import ml_dtypes

import concourse.bass as bass
import concourse.bacc as bacc
import concourse.mybir as mybir
import concourse.tile as tile
from concourse import bass_utils
from concourse.masks import make_identity

N_CORES = 8
B = 128
N = 512
F = 128
G = B // N_CORES          # graphs per core
NB = N // 128             # node blocks per graph
P = 128

F32 = mybir.dt.float32
BF16 = mybir.dt.bfloat16

_cache = {}
last_results = None


def _build():
    nc = bacc.Bacc("TRN2", target_bir_lowering=False, debug=False,
                   num_devices=N_CORES)
    att = nc.dram_tensor("att", [G, P, NB * N], BF16,
                         kind="ExternalInput").ap()
    xt = nc.dram_tensor("xt", [G, P, NB * F], BF16,
                        kind="ExternalInput").ap()
    w1 = nc.dram_tensor("w1", [F, F], F32, kind="ExternalInput").ap()
    w2 = nc.dram_tensor("w2", [F, F], F32, kind="ExternalInput").ap()
    b1 = nc.dram_tensor("b1", [F, 1], F32, kind="ExternalInput").ap()
    b2 = nc.dram_tensor("b2", [F, 1], F32, kind="ExternalInput").ap()
    wr1 = nc.dram_tensor("wr1", [F, 64], F32, kind="ExternalInput").ap()
    br1 = nc.dram_tensor("br1", [64, 1], F32, kind="ExternalInput").ap()
    wr2 = nc.dram_tensor("wr2", [64, 1], F32, kind="ExternalInput").ap()
    br2 = nc.dram_tensor("br2", [1, 1], F32, kind="ExternalInput").ap()
    out = nc.dram_tensor("out", [1, G], F32, kind="ExternalOutput").ap()

    with tile.TileContext(nc) as tc:
        with tc.tile_pool(name="const", bufs=1) as cp, \
             tc.tile_pool(name="sb_big", bufs=4) as bp, \
             tc.tile_pool(name="sb_med", bufs=4) as mp, \
             tc.tile_pool(name="sb_zb", bufs=8) as zp, \
             tc.tile_pool(name="sb_small", bufs=3) as sp, \
             tc.tile_pool(name="psum_v", bufs=3, space="PSUM") as pv, \
             tc.tile_pool(name="psum_t", bufs=2, space="PSUM") as pt, \
             tc.tile_pool(name="psum_p", bufs=2, space="PSUM") as pp, \
             tc.tile_pool(name="psum_s", bufs=1, space="PSUM") as ps:

            # ---- constants / weights ----
            ident = cp.tile([P, P], F32)
            make_identity(nc, ident[:])
            identb = cp.tile([P, P], BF16)
            nc.gpsimd.tensor_copy(identb[:], ident[:])
            ones_col = cp.tile([P, 1], BF16)
            nc.vector.memset(ones_col[:], 1.0)

            w1f = cp.tile([P, P], F32)
            w2f = cp.tile([P, P], F32)
            nc.sync.dma_start(out=w1f[:], in_=w1[:])
            nc.sync.dma_start(out=w2f[:], in_=w2[:])
            w1b = cp.tile([P, P], BF16)
            w2b = cp.tile([P, P], BF16)
            nc.gpsimd.tensor_copy(w1b[:], w1f[:])
            nc.gpsimd.tensor_copy(w2b[:], w2f[:])
            b1f = cp.tile([P, 1], F32)
            b2f = cp.tile([P, 1], F32)
            nc.sync.dma_start(out=b1f[:], in_=b1[:])
            nc.sync.dma_start(out=b2f[:], in_=b2[:])
            wr1f = cp.tile([P, 64], F32)
            br1f = cp.tile([64, 1], F32)
            wr2f = cp.tile([64, 1], F32)
            br2f = cp.tile([1, 1], F32)
            nc.sync.dma_start(out=wr1f[:], in_=wr1[:])
            nc.sync.dma_start(out=br1f[:], in_=br1[:])
            nc.sync.dma_start(out=wr2f[:], in_=wr2[:])
            nc.sync.dma_start(out=br2f[:], in_=br2[:])

            h_all = cp.tile([P, G], F32)

            def graph_pipeline(g):
                # ---- load A^T (bf16, partition-contiguous) ----
                atb = bp.tile([P, NB * N], BF16, tag="atb")
                nc.sync.dma_start(out=atb[:], in_=att[g])

                # ---- degree: deg_row[1, i] = sum_j A^T[j, i] ----
                deg_psum = ps.tile([1, N], F32, space="PSUM", tag="sp")
                for jb in range(NB):
                    nc.tensor.matmul(out=deg_psum[:],
                                     lhsT=ones_col[:],
                                     rhs=atb[:, jb * N:(jb + 1) * N],
                                     start=(jb == 0), stop=(jb == NB - 1))
                degrow = sp.tile([1, N], F32, tag="degrow")
                nc.scalar.copy(degrow[:], deg_psum[:])
                # reshape [1, 512] -> [4, 128] via sbuf-to-sbuf DMA
                deg4 = sp.tile([NB, P], F32, tag="deg4")
                nc.gpsimd.dma_start(out=deg4[:], in_=degrow[:])

                # dinv = (deg > 0) * rsqrt(max(deg, 1))
                mask4 = sp.tile([NB, P], F32, tag="mask4")
                nc.vector.tensor_scalar(out=mask4[:], in0=deg4[:],
                                        scalar1=0.5, scalar2=None,
                                        op0=mybir.AluOpType.is_ge)
                degc = sp.tile([NB, P], F32, tag="degc")
                nc.vector.tensor_scalar_max(out=degc[:], in0=deg4[:],
                                            scalar1=1.0)
                s4 = sp.tile([NB, P], F32, tag="s4")
                nc.scalar.sqrt(s4[:], degc[:])
                r4 = sp.tile([NB, P], F32, tag="r4")
                nc.vector.reciprocal(r4[:], s4[:])
                dinv4 = sp.tile([NB, P], F32, tag="dinv4")
                nc.vector.tensor_mul(dinv4[:], r4[:], mask4[:])
                dinvrow = sp.tile([1, N], F32, tag="dinvrow")
                nc.gpsimd.dma_start(out=dinvrow[:], in_=dinv4[:])
                dinvrb = sp.tile([1, N], BF16, tag="dinvrb")
                nc.scalar.copy(dinvrb[:], dinvrow[:])

                # ---- Sb = bf16( dinv_j * A^T * dinv_i ) ----
                Sb = bp.tile([P, NB * N], BF16, tag="Sb")
                for jb in range(NB):
                    outer = pp.tile([P, N], F32, space="PSUM", tag="pp")
                    nc.tensor.matmul(out=outer[:],
                                     lhsT=dinvrb[:, jb * P:(jb + 1) * P],
                                     rhs=dinvrb[:],
                                     start=True, stop=True)
                    nc.vector.tensor_tensor(
                        out=Sb[:, jb * N:(jb + 1) * N],
                        in0=atb[:, jb * N:(jb + 1) * N],
                        in1=outer[:], op=mybir.AluOpType.mult)

                # ---- node features ----
                xs = bp.tile([P, NB * F], BF16, tag="xs")
                nc.sync.dma_start(out=xs[:], in_=xt[g])
                yield

                def apply_S(zb):
                    tk = pv.tile([P, NB * F], F32, space="PSUM", tag="pv")
                    for ib in range(NB):
                        for jb in range(NB):
                            nc.tensor.matmul(
                                out=tk[:, ib * F:(ib + 1) * F],
                                lhsT=Sb[:, jb * N + ib * P:
                                        jb * N + (ib + 1) * P],
                                rhs=zb[:, jb * F:(jb + 1) * F],
                                start=(jb == 0), stop=(jb == NB - 1))
                    return tk

                x_cur = xs
                for layer, (wb, bf) in enumerate(((w1b, b1f), (w2b, b2f))):
                    # factored poly3: acc = (I + S^2)(I + S) x
                    t1 = apply_S(x_cur)
                    yield
                    y1 = zp.tile([P, NB * F], BF16, tag="zb")
                    nc.vector.tensor_tensor(out=y1[:], in0=x_cur[:],
                                            in1=t1[:],
                                            op=mybir.AluOpType.add)
                    w_ = apply_S(y1)
                    yield
                    wb16 = zp.tile([P, NB * F], BF16, tag="zb")
                    nc.scalar.copy(wb16[:], w_[:])
                    tk = apply_S(wb16)
                    yield
                    acc = mp.tile([P, NB * F], BF16, tag="acc")
                    nc.vector.tensor_tensor(out=acc[:], in0=y1[:],
                                            in1=tk[:],
                                            op=mybir.AluOpType.add)

                    # proj: xT_next[o, node] = relu(W.T @ acc.T + b)
                    accT = pt.tile([P, NB * F], BF16, space="PSUM", tag="pt")
                    for nb_ in range(NB):
                        nc.tensor.transpose(
                            out=accT[:, nb_ * P:(nb_ + 1) * P],
                            in_=acc[:, nb_ * F:(nb_ + 1) * F],
                            identity=identb[:])
                    accTb = mp.tile([P, NB * F], BF16, tag="accTb")
                    nc.scalar.copy(accTb[:], accT[:])
                    pj = pp.tile([P, NB * F], F32, space="PSUM", tag="pp")
                    nc.tensor.matmul(out=pj[:], lhsT=wb[:], rhs=accTb[:],
                                     start=True, stop=True)
                    if layer == 0:
                        xTb = mp.tile([P, NB * F], BF16, tag="xT")
                        nc.scalar.activation(
                            out=xTb[:], in_=pj[:],
                            func=mybir.ActivationFunctionType.Relu,
                            bias=bf[:], scale=1.0)
                        # transpose back to [node, f] for layer 2
                        x2p = pt.tile([P, NB * F], BF16, space="PSUM", tag="pt")
                        for nb_ in range(NB):
                            nc.tensor.transpose(
                                out=x2p[:, nb_ * P:(nb_ + 1) * P],
                                in_=xTb[:, nb_ * P:(nb_ + 1) * P],
                                identity=identb[:])
                        x_cur = mp.tile([P, NB * F], BF16, tag="xs2")
                        nc.scalar.copy(x_cur[:], x2p[:])
                        yield
                    else:
                        # relu + free mean-pool over nodes via accum_out
                        xT = mp.tile([P, NB * F], F32, tag="xT")
                        nc.scalar.activation(
                            out=xT[:], in_=pj[:],
                            func=mybir.ActivationFunctionType.Relu,
                            bias=bf[:], scale=1.0,
                            accum_out=h_all[:, g:g + 1])
                yield

            # run graphs in interleaved pairs so PE always has
            # independent matmul work during cross-engine dependency hops
            W = 3
            for base in range(0, G, W):
                gens = [graph_pipeline(base + i)
                        for i in range(min(W, G - base))]
                alive = list(gens)
                while alive:
                    for gen in list(alive):
                        try:
                            next(gen)
                        except StopIteration:
                            alive.remove(gen)

            # ---- head MLP over all 16 graphs ----
            hsc = cp.tile([P, G], F32)
            nc.scalar.mul(hsc[:], h_all[:], 1.0 / N)
            q1 = ps.tile([64, G], F32, space="PSUM", tag="sp")
            nc.tensor.matmul(out=q1[:], lhsT=wr1f[:], rhs=hsc[:],
                             start=True, stop=True)
            q1s = cp.tile([64, G], F32)
            nc.scalar.activation(out=q1s[:], in_=q1[:],
                                 func=mybir.ActivationFunctionType.Relu,
                                 bias=br1f[:], scale=1.0)
            q2 = ps.tile([1, G], F32, space="PSUM", tag="sp")
            nc.tensor.matmul(out=q2[:], lhsT=wr2f[:], rhs=q1s[:],
                             start=True, stop=True)
            outs = cp.tile([1, G], F32)
            nc.scalar.activation(out=outs[:], in_=q2[:],
                                 func=mybir.ActivationFunctionType.Identity,
                                 bias=br2f[:], scale=1.0)
            nc.sync.dma_start(out=out[:], in_=outs[:])

    nc.compile()
    return nc


def kernel(**inputs):
    global last_results
    X = np.asarray(inputs["X"], dtype=np.float32)
    edge_index = np.asarray(inputs["edge_index"])
    W1 = np.asarray(inputs["W1"], dtype=np.float32)
    b1 = np.asarray(inputs["b1"], dtype=np.float32)
    W2 = np.asarray(inputs["W2"], dtype=np.float32)
    b2 = np.asarray(inputs["b2"], dtype=np.float32)
    Wr1 = np.asarray(inputs["Wr1"], dtype=np.float32)
    br1 = np.asarray(inputs["br1"], dtype=np.float32)
    Wr2 = np.asarray(inputs["Wr2"], dtype=np.float32)
    br2 = np.asarray(inputs["br2"], dtype=np.float32)

    # ---- shard: dense transposed adjacency layout + node blocks ----
    src = edge_index[0].astype(np.int64)
    dst = edge_index[1].astype(np.int64)
    valid = (src >= 0) & (src < B * N) & (dst >= 0) & (dst < B * N)
    src = src[valid]
    dst = dst[valid]
    # att[g, p, b*N + i] = 1 iff edge i->j with j = b*128 + p  (bf16 bits)
    at16 = np.zeros((B, P, NB * N), dtype=np.uint16)
    at16[src >> 9, dst & (P - 1), ((dst >> 7) & 3) * N + (src & (N - 1))] = \
        0x3F80  # 1.0 in bf16
    atb = at16.view(ml_dtypes.bfloat16)
    # xt[g, p, b*F + f] = X[g*N + b*128 + p, f]
    xg = np.ascontiguousarray(
        X.reshape(B, NB, P, F).transpose(0, 2, 1, 3)).reshape(
            B, P, NB * F).astype(ml_dtypes.bfloat16)

    b1c = np.ascontiguousarray(b1.reshape(F, 1))
    b2c = np.ascontiguousarray(b2.reshape(F, 1))
    br1c = np.ascontiguousarray(br1.reshape(64, 1))
    br2c = np.ascontiguousarray(br2.reshape(1, 1))

    in_maps = []
    for c in range(N_CORES):
        in_maps.append({
            "att": np.ascontiguousarray(atb[c * G:(c + 1) * G]),
            "xt": np.ascontiguousarray(xg[c * G:(c + 1) * G]),
            "w1": W1, "w2": W2, "b1": b1c, "b2": b2c,
            "wr1": Wr1, "br1": br1c, "wr2": Wr2, "br2": br2c,
        })

    if "nc" not in _cache:
        _cache["nc"] = _build()
    nc = _cache["nc"]

    res = bass_utils.run_bass_kernel_spmd(
        nc, in_maps, core_ids=list(range(N_CORES)),
        trace=bool(os.environ.get("GNN_TRACE")))
    last_results = res

    out = np.concatenate(
        [res.results[c]["out"].reshape(-1) for c in range(N_CORES)])
    return out.astype(np.float32)


# revision 15
# speedup vs baseline: 1.1587x; 1.1587x over previous
"""Trainium2 Bass kernel for GeneralPolyGNN (B=128 graphs, N=512 nodes, F=128).

Model:  S = D^-1/2 A D^-1/2 (per-graph dense GSO from edge_index)
        x = relu(poly3(S, x) @ W1 + b1); x = relu(poly3(S, x) @ W2 + b2)
        h = mean_nodes(x); out = relu(h @ Wr1 + br1) @ Wr2 + br2
        where poly3(S, x) = x + Sx + S^2x + S^3x  (Horner: x + S(x + S(x + Sx)))

Sharding: data-parallel over graphs — 16 graphs per core on 8 NeuronCores.
The host shards node features and lays each graph's edge set out as its dense
0/1 transposed-adjacency block (partition-contiguous bf16); all numeric work
(degrees, normalization, polynomial matmul chain, activations, pooling, head
MLP) runs on device.

Device layout (per graph g):
  att[g, p, b*512 + i] = 1 iff edge i -> j, j = b*128 + p      (A^T as bf16)
  x tiles: [128 part, (4 node-blocks x 128 feat)], node = blk*128 + p
  Sb = bf16(dinv_j * A^T * dinv_i): scaled GSO transpose, built on device
  apply: t[ib] += Sb[jb, ib-slice].T @ z[jb]   (PSUM f32, 16 matmuls)
"""

import os
import numpy as np
import ml_dtypes

import concourse.bass as bass
import concourse.bacc as bacc
import concourse.mybir as mybir
import concourse.tile as tile
from concourse import bass_utils
from concourse.masks import make_identity

N_CORES = 8
B = 128
N = 512
F = 128
G = B // N_CORES          # graphs per core
NB = N // 128             # node blocks per graph
P = 128

F32 = mybir.dt.float32
BF16 = mybir.dt.bfloat16

_cache = {}
last_results = None


def _build():
    nc = bacc.Bacc("TRN2", target_bir_lowering=False, debug=False,
                   num_devices=N_CORES)
    att = nc.dram_tensor("att", [G, P, NB * N], BF16,
                         kind="ExternalInput").ap()
    xt = nc.dram_tensor("xt", [G, P, NB * F], BF16,
                        kind="ExternalInput").ap()
    w1 = nc.dram_tensor("w1", [F, F], F32, kind="ExternalInput").ap()
    w2 = nc.dram_tensor("w2", [F, F], F32, kind="ExternalInput").ap()
    b1 = nc.dram_tensor("b1", [F, 1], F32, kind="ExternalInput").ap()
    b2 = nc.dram_tensor("b2", [F, 1], F32, kind="ExternalInput").ap()
    wr1 = nc.dram_tensor("wr1", [F, 64], F32, kind="ExternalInput").ap()
    br1 = nc.dram_tensor("br1", [64, 1], F32, kind="ExternalInput").ap()
    wr2 = nc.dram_tensor("wr2", [64, 1], F32, kind="ExternalInput").ap()
    br2 = nc.dram_tensor("br2", [1, 1], F32, kind="ExternalInput").ap()
    out = nc.dram_tensor("out", [1, G], F32, kind="ExternalOutput").ap()

    with tile.TileContext(nc) as tc:
        with tc.tile_pool(name="const", bufs=1) as cp, \
             tc.tile_pool(name="sb_big", bufs=5) as bp, \
             tc.tile_pool(name="sb_med", bufs=4) as mp, \
             tc.tile_pool(name="sb_zb", bufs=8) as zp, \
             tc.tile_pool(name="sb_small", bufs=5) as sp, \
             tc.tile_pool(name="psum_v", bufs=3, space="PSUM") as pv, \
             tc.tile_pool(name="psum_t", bufs=2, space="PSUM") as pt, \
             tc.tile_pool(name="psum_p", bufs=2, space="PSUM") as pp, \
             tc.tile_pool(name="psum_s", bufs=1, space="PSUM") as ps:

            # ---- constants / weights ----
            ident = cp.tile([P, P], F32)
            make_identity(nc, ident[:])
            identb = cp.tile([P, P], BF16)
            nc.gpsimd.tensor_copy(identb[:], ident[:])
            ones_col = cp.tile([P, 1], BF16)
            nc.vector.memset(ones_col[:], 1.0)

            w1f = cp.tile([P, P], F32)
            w2f = cp.tile([P, P], F32)
            nc.sync.dma_start(out=w1f[:], in_=w1[:])
            nc.sync.dma_start(out=w2f[:], in_=w2[:])
            w1b = cp.tile([P, P], BF16)
            w2b = cp.tile([P, P], BF16)
            nc.gpsimd.tensor_copy(w1b[:], w1f[:])
            nc.gpsimd.tensor_copy(w2b[:], w2f[:])
            b1f = cp.tile([P, 1], F32)
            b2f = cp.tile([P, 1], F32)
            nc.sync.dma_start(out=b1f[:], in_=b1[:])
            nc.sync.dma_start(out=b2f[:], in_=b2[:])
            wr1f = cp.tile([P, 64], F32)
            br1f = cp.tile([64, 1], F32)
            wr2f = cp.tile([64, 1], F32)
            br2f = cp.tile([1, 1], F32)
            nc.sync.dma_start(out=wr1f[:], in_=wr1[:])
            nc.sync.dma_start(out=br1f[:], in_=br1[:])
            nc.sync.dma_start(out=wr2f[:], in_=wr2[:])
            nc.sync.dma_start(out=br2f[:], in_=br2[:])

            h_all = cp.tile([P, G], F32)

            def graph_pipeline(g):
                # ---- load A^T (bf16, partition-contiguous) ----
                atb = bp.tile([P, NB * N], BF16, tag="atb")
                nc.sync.dma_start(out=atb[:], in_=att[g])

                # ---- degree: deg_row[1, i] = sum_j A^T[j, i] ----
                deg_psum = ps.tile([1, N], F32, space="PSUM", tag="sp")
                for jb in range(NB):
                    nc.tensor.matmul(out=deg_psum[:],
                                     lhsT=ones_col[:],
                                     rhs=atb[:, jb * N:(jb + 1) * N],
                                     start=(jb == 0), stop=(jb == NB - 1))
                degrow = sp.tile([1, N], F32, tag="degrow")
                nc.scalar.copy(degrow[:], deg_psum[:])
                # reshape [1, 512] -> [4, 128] via sbuf-to-sbuf DMA
                deg4 = sp.tile([NB, P], F32, tag="deg4")
                nc.gpsimd.dma_start(out=deg4[:], in_=degrow[:])

                # dinv = (deg > 0) * rsqrt(max(deg, 1))
                mask4 = sp.tile([NB, P], F32, tag="mask4")
                nc.vector.tensor_scalar(out=mask4[:], in0=deg4[:],
                                        scalar1=0.5, scalar2=None,
                                        op0=mybir.AluOpType.is_ge)
                degc = sp.tile([NB, P], F32, tag="degc")
                nc.vector.tensor_scalar_max(out=degc[:], in0=deg4[:],
                                            scalar1=1.0)
                s4 = sp.tile([NB, P], F32, tag="s4")
                nc.scalar.sqrt(s4[:], degc[:])
                r4 = sp.tile([NB, P], F32, tag="r4")
                nc.vector.reciprocal(r4[:], s4[:])
                dinv4 = sp.tile([NB, P], F32, tag="dinv4")
                nc.vector.tensor_mul(dinv4[:], r4[:], mask4[:])
                dinvrow = sp.tile([1, N], F32, tag="dinvrow")
                nc.gpsimd.dma_start(out=dinvrow[:], in_=dinv4[:])
                dinvrb = sp.tile([1, N], BF16, tag="dinvrb")
                nc.scalar.copy(dinvrb[:], dinvrow[:])

                # ---- Sb = bf16( dinv_j * A^T * dinv_i ) ----
                Sb = bp.tile([P, NB * N], BF16, tag="Sb")
                for jb in range(NB):
                    outer = pp.tile([P, N], F32, space="PSUM", tag="pp")
                    nc.tensor.matmul(out=outer[:],
                                     lhsT=dinvrb[:, jb * P:(jb + 1) * P],
                                     rhs=dinvrb[:],
                                     start=True, stop=True)
                    nc.vector.tensor_tensor(
                        out=Sb[:, jb * N:(jb + 1) * N],
                        in0=atb[:, jb * N:(jb + 1) * N],
                        in1=outer[:], op=mybir.AluOpType.mult)

                # ---- node features ----
                xs = bp.tile([P, NB * F], BF16, tag="xs")
                nc.sync.dma_start(out=xs[:], in_=xt[g])
                yield

                def apply_S(zb):
                    tk = pv.tile([P, NB * F], F32, space="PSUM", tag="pv")
                    for ib in range(NB):
                        for jb in range(NB):
                            nc.tensor.matmul(
                                out=tk[:, ib * F:(ib + 1) * F],
                                lhsT=Sb[:, jb * N + ib * P:
                                        jb * N + (ib + 1) * P],
                                rhs=zb[:, jb * F:(jb + 1) * F],
                                start=(jb == 0), stop=(jb == NB - 1))
                    return tk

                x_cur = xs
                for layer, (wb, bf) in enumerate(((w1b, b1f), (w2b, b2f))):
                    # factored poly3: acc = (I + S^2)(I + S) x
                    t1 = apply_S(x_cur)
                    yield
                    y1 = zp.tile([P, NB * F], BF16, tag="zb")
                    nc.vector.tensor_tensor(out=y1[:], in0=x_cur[:],
                                            in1=t1[:],
                                            op=mybir.AluOpType.add)
                    w_ = apply_S(y1)
                    yield
                    wb16 = zp.tile([P, NB * F], BF16, tag="zb")
                    nc.scalar.copy(wb16[:], w_[:])
                    tk = apply_S(wb16)
                    yield
                    acc = mp.tile([P, NB * F], BF16, tag="acc")
                    nc.vector.tensor_tensor(out=acc[:], in0=y1[:],
                                            in1=tk[:],
                                            op=mybir.AluOpType.add)

                    # proj: xT_next[o, node] = relu(W.T @ acc.T + b)
                    accT = pt.tile([P, NB * F], BF16, space="PSUM", tag="pt")
                    for nb_ in range(NB):
                        nc.tensor.transpose(
                            out=accT[:, nb_ * P:(nb_ + 1) * P],
                            in_=acc[:, nb_ * F:(nb_ + 1) * F],
                            identity=identb[:])
                    accTb = mp.tile([P, NB * F], BF16, tag="accTb")
                    nc.scalar.copy(accTb[:], accT[:])
                    pj = pp.tile([P, NB * F], F32, space="PSUM", tag="pp")
                    nc.tensor.matmul(out=pj[:], lhsT=wb[:], rhs=accTb[:],
                                     start=True, stop=True)
                    if layer == 0:
                        xTb = mp.tile([P, NB * F], BF16, tag="xT")
                        nc.scalar.activation(
                            out=xTb[:], in_=pj[:],
                            func=mybir.ActivationFunctionType.Relu,
                            bias=bf[:], scale=1.0)
                        # transpose back to [node, f] for layer 2
                        x2p = pt.tile([P, NB * F], BF16, space="PSUM", tag="pt")
                        for nb_ in range(NB):
                            nc.tensor.transpose(
                                out=x2p[:, nb_ * P:(nb_ + 1) * P],
                                in_=xTb[:, nb_ * P:(nb_ + 1) * P],
                                identity=identb[:])
                        x_cur = mp.tile([P, NB * F], BF16, tag="xs2")
                        nc.scalar.copy(x_cur[:], x2p[:])
                        yield
                    else:
                        # relu + free mean-pool over nodes via accum_out
                        xT = mp.tile([P, NB * F], F32, tag="xT")
                        nc.scalar.activation(
                            out=xT[:], in_=pj[:],
                            func=mybir.ActivationFunctionType.Relu,
                            bias=bf[:], scale=1.0,
                            accum_out=h_all[:, g:g + 1])
                yield

            # Interleave graphs so PE always has independent matmul work
            # during cross-engine dependency hops. Keep 2 graphs in their
            # main phase and prefetch the prep chunk (adjacency load,
            # degree, dinv, Sb build) of 2 more.
            gens = [graph_pipeline(g) for g in range(G)]
            started = 0
            PREFETCH = 4
            W = 2
            for _ in range(min(PREFETCH, G)):
                next(gens[started])
                started += 1
            active = gens[:W]
            next_active = W
            while active:
                for gen in list(active):
                    try:
                        next(gen)
                    except StopIteration:
                        active.remove(gen)
                        if next_active < G:
                            if next_active >= started:
                                next(gens[next_active])
                                started += 1
                            active.append(gens[next_active])
                            next_active += 1
                        if started < G:
                            next(gens[started])
                            started += 1

            # ---- head MLP over all 16 graphs ----
            hsc = cp.tile([P, G], F32)
            nc.scalar.mul(hsc[:], h_all[:], 1.0 / N)
            q1 = ps.tile([64, G], F32, space="PSUM", tag="sp")
            nc.tensor.matmul(out=q1[:], lhsT=wr1f[:], rhs=hsc[:],
                             start=True, stop=True)
            q1s = cp.tile([64, G], F32)
            nc.scalar.activation(out=q1s[:], in_=q1[:],
                                 func=mybir.ActivationFunctionType.Relu,
                                 bias=br1f[:], scale=1.0)
            q2 = ps.tile([1, G], F32, space="PSUM", tag="sp")
            nc.tensor.matmul(out=q2[:], lhsT=wr2f[:], rhs=q1s[:],
                             start=True, stop=True)
            outs = cp.tile([1, G], F32)
            nc.scalar.activation(out=outs[:], in_=q2[:],
                                 func=mybir.ActivationFunctionType.Identity,
                                 bias=br2f[:], scale=1.0)
            nc.sync.dma_start(out=out[:], in_=outs[:])

    nc.compile()
    return nc


def kernel(**inputs):
    global last_results
    X = np.asarray(inputs["X"], dtype=np.float32)
    edge_index = np.asarray(inputs["edge_index"])
    W1 = np.asarray(inputs["W1"], dtype=np.float32)
    b1 = np.asarray(inputs["b1"], dtype=np.float32)
    W2 = np.asarray(inputs["W2"], dtype=np.float32)
    b2 = np.asarray(inputs["b2"], dtype=np.float32)
    Wr1 = np.asarray(inputs["Wr1"], dtype=np.float32)
    br1 = np.asarray(inputs["br1"], dtype=np.float32)
    Wr2 = np.asarray(inputs["Wr2"], dtype=np.float32)
    br2 = np.asarray(inputs["br2"], dtype=np.float32)

    # ---- shard: dense transposed adjacency layout + node blocks ----
    src = edge_index[0].astype(np.int64)
    dst = edge_index[1].astype(np.int64)
    valid = (src >= 0) & (src < B * N) & (dst >= 0) & (dst < B * N)
    src = src[valid]
    dst = dst[valid]
    # att[g, p, b*N + i] = 1 iff edge i->j with j = b*128 + p  (bf16 bits)
    at16 = np.zeros((B, P, NB * N), dtype=np.uint16)
    at16[src >> 9, dst & (P - 1), ((dst >> 7) & 3) * N + (src & (N - 1))] = \
        0x3F80  # 1.0 in bf16
    atb = at16.view(ml_dtypes.bfloat16)
    # xt[g, p, b*F + f] = X[g*N + b*128 + p, f]
    xg = np.ascontiguousarray(
        X.reshape(B, NB, P, F).transpose(0, 2, 1, 3)).reshape(
            B, P, NB * F).astype(ml_dtypes.bfloat16)

    b1c = np.ascontiguousarray(b1.reshape(F, 1))
    b2c = np.ascontiguousarray(b2.reshape(F, 1))
    br1c = np.ascontiguousarray(br1.reshape(64, 1))
    br2c = np.ascontiguousarray(br2.reshape(1, 1))

    in_maps = []
    for c in range(N_CORES):
        in_maps.append({
            "att": np.ascontiguousarray(atb[c * G:(c + 1) * G]),
            "xt": np.ascontiguousarray(xg[c * G:(c + 1) * G]),
            "w1": W1, "w2": W2, "b1": b1c, "b2": b2c,
            "wr1": Wr1, "br1": br1c, "wr2": Wr2, "br2": br2c,
        })

    if "nc" not in _cache:
        _cache["nc"] = _build()
    nc = _cache["nc"]

    res = bass_utils.run_bass_kernel_spmd(
        nc, in_maps, core_ids=list(range(N_CORES)),
        trace=bool(os.environ.get("GNN_TRACE")))
    last_results = res

    out = np.concatenate(
        [res.results[c]["out"].reshape(-1) for c in range(N_CORES)])
    return out.astype(np.float32)
